# revision 5
# baseline (speedup 1.0000x reference)
"""Trainium2 Bass kernel for nn_Attention_5927054869144.

Channel-attention over [B=8, C=64, H=256, W=256] inputs. Data-parallel over
batch: one batch element per NeuronCore (8 cores), no collectives.

Per-core pipeline (x_b viewed as [64, 65536], 8 spatial blocks of 8192):
  1. x loaded fp32 via HWDGE (sync/scalar) in 8 chunks of [65, 8, 1024]
     (4KB descriptors, no dtype cast in DMA).
  2. qkvT projection with x-chunk stationary on the PE in float32r (FP22,
     full rate at N=256: 192 real columns + 64 pad) -> q/k/v in
     spatial-partition layout, evacuated fp32->fp16 into (r, h, alpha, i)
     ordered slots (softmax scale folded into Wq/bq host-side).
  3. Per-head-pair dots matmuls accumulated in PSUM over all 64 spatial tile
     groups; v transposed to dim-partition layout via TensorE transposes.
  4. Batched max-free softmax: one fp32 Exp over all heads (|dots| << 88 so
     no overflow), one reduce_sum, one reciprocal, folded into per-head
     copies of Wo^T; tiny mh matmuls.
  5. Final output = mh^T @ v_dp per 512-column chunk (fp16 operands),
     evacuated fp32 and DMAed per 4096-column chunk on sync/scalar HWDGE.
"""

import os
import sys

import numpy as np

for _p in ("/opt/trn_rl_repo", "/root/.axon_site/_ro/trn_rl_repo"):
    if os.path.isdir(_p) and _p not in sys.path:
        sys.path.insert(0, _p)

from concourse import bacc, mybir, tile  # noqa: E402
from concourse.bass_utils import run_bass_kernel_spmd  # noqa: E402

F32 = mybir.dt.float32
F32R = mybir.dt.float32r
F16 = mybir.dt.float16

HEADS = 8
C = 64
HW = 65536          # 256*256 spatial positions per batch element
BL = HW // HEADS    # 8192, per-head block length
NE = 8              # input chunks (intra-block column ranges)
CHUNK_B = 4096      # phase-B output chunk columns per head

LAST_RESULTS = None


def _cp(eng, out, in_):
    if hasattr(eng, "tensor_copy"):
        eng.tensor_copy(out, in_)
    else:
        eng.copy(out, in_)


def _build_kernel(hw=HW):
    bl = hw // HEADS
    el = bl // NE           # 1024 intra-block cols per chunk
    tpe = el // 128         # 8 tile groups per chunk
    n_groups = bl // 128    # 64 total tile groups
    chunk_b = min(CHUNK_B, bl)
    s5n = chunk_b // 512
    c0n = bl // chunk_b

    nc = bacc.Bacc("TRN2", target_bir_lowering=False, debug=False)
    x_d = nc.dram_tensor("x", [65, hw], F32, kind="ExternalInput")
    wqkv_d = nc.dram_tensor("wqkv", [65, 256], F32, kind="ExternalInput")
    wot_d = nc.dram_tensor("wot", [128, 64], F32, kind="ExternalInput")
    ident_d = nc.dram_tensor("ident", [128, 128], F16, kind="ExternalInput")
    out_d = nc.dram_tensor("out", [64, hw], F32, kind="ExternalOutput")

    x_ap = x_d.ap()
    out_ap = out_d.ap()
    # [pr, s, o, n] view of the output: head h = 2*pr + s
    out_v = out_ap.rearrange("o (p s n) -> p s o n", p=4, s=2)

    with tile.TileContext(nc) as tc:
        with (
            tc.tile_pool(name="consts", bufs=1) as cpool,
            tc.tile_pool(name="pers", bufs=1) as pers,
            tc.tile_pool(name="dotsp", bufs=1, space="PSUM") as dotspool,
        ):
            wqkv_sb = cpool.tile([65, 256], F32R)
            wot_sb = cpool.tile([128, 64], F32)
            ident_sb = cpool.tile([128, 128], F16)
            nc.sync.dma_start(out=wqkv_sb[:, :],
                              in_=wqkv_d.ap()[:, :].bitcast(F32R))
            nc.scalar.dma_start(out=wot_sb[:, :], in_=wot_d.ap()[:, :])
            nc.sync.dma_start(out=ident_sb[:, :], in_=ident_d.ap()[:, :])

            # v in dim-partition layout: [d_even(0:64)|d_odd(64:128), pair, n]
            vdp = pers.tile([128, 4 * bl], F16)
            vdp_v = vdp.rearrange("p (r n) -> p r n", r=4)
            dots_ps = [
                dotspool.tile([128, 128], F32, name=f"dots{p}") for p in range(4)
            ]

            # ---------------- Phase A ----------------
            with (
                tc.tile_pool(name="xq", bufs=2) as xpool,
                tc.tile_pool(name="slots", bufs=5) as slotpool,
                tc.tile_pool(name="projp", bufs=2, space="PSUM") as projpool,
                tc.tile_pool(name="vtrp", bufs=2, space="PSUM") as vtrpool,
            ):
                x_blk = x_ap.rearrange("p (i n) -> p i n", i=8)
                slots = {}

                def consume(g):
                    # dots + v-transpose for a group whose slot is fully evac'd
                    slot = slots.pop(g)
                    vt = vtrpool.tile([128, 512], F16, name="vt")
                    for pr in range(4):
                        qs = slot[:, 128 * pr: 128 * pr + 128]
                        ks = slot[:, 512 + 128 * pr: 512 + 128 * pr + 128]
                        vs = slot[:, 1024 + 128 * pr: 1024 + 128 * pr + 128]
                        nc.tensor.matmul(
                            dots_ps[pr][:, :],
                            lhsT=qs,
                            rhs=ks,
                            start=(g == 0),
                            stop=(g == n_groups - 1),
                        )
                        nc.tensor.transpose(
                            vt[:, pr * 128:(pr + 1) * 128], vs, ident_sb[:, :]
                        )
                    voff = g * 128
                    ve = nc.vector if g % 2 == 0 else nc.scalar
                    _cp(ve, vdp_v[:, :, voff:voff + 128], vt[:, :])

                for e in range(NE):
                    xe = xpool.tile([65, 8 * el], F32R, name="xe")
                    xe_v = xe.rearrange("p (i n) -> p i n", i=8)
                    deng = nc.sync if e % 2 == 0 else nc.scalar
                    deng.dma_start(
                        out=xe_v,
                        in_=x_blk[:, :, e * el:(e + 1) * el].bitcast(F32R),
                    )
                    for t0 in range(tpe):
                        g = e * tpe + t0
                        # slot cols: r*512 + h*64 + alpha*8 + i
                        slot = slotpool.tile([128, 1536], F16, name="slot")
                        slot_ic = slot.rearrange("p (c i) -> p i c", i=8)
                        slots[g] = slot
                        for ip in range(4):  # block pairs (2i, 2i+1)
                            pp = projpool.tile([128, 512], F32, name="pp")
                            for cc in range(2):
                                i = 2 * ip + cc
                                nc.tensor.matmul(
                                    pp[:, cc * 256:(cc + 1) * 256],
                                    lhsT=xe_v[:, i, t0 * 128:
                                              t0 * 128 + 128],
                                    rhs=wqkv_sb[:, :],
                                    start=True,
                                    stop=True,
                                )
                            src = pp.rearrange(
                                "p (i2 c) -> p i2 c", i2=2)[:, :, 0:192]
                            dst = slot_ic[:, 2 * ip:2 * ip + 2, :]
                            ce = nc.vector if ip % 2 == 0 else nc.scalar
                            _cp(ce, dst, src)
                        if g >= 3:
                            consume(g - 3)
                for g in (n_groups - 3, n_groups - 2, n_groups - 1):
                    consume(g)

            # ---------------- Softmax + output ----------------
            with (
                tc.tile_pool(name="smx", bufs=1) as smx,
                tc.tile_pool(name="mhp", bufs=1, space="PSUM") as mhpool,
                tc.tile_pool(name="finp", bufs=3, space="PSUM") as finpool,
                tc.tile_pool(name="outs", bufs=3) as outpool,
            ):
                ds_sb = smx.tile([128, 512], F32)
                shif = smx.tile([128, 512], F32)
                negmax = smx.tile([128, 8], F32)
                exps = smx.tile([128, 512], F16)
                rowsum = smx.tile([128, 8], F32)
                recip = smx.tile([128, 8], F32)
                wots = smx.tile([128, 512], F16)
                mh_sb = smx.tile([128, 256], F16)
                mh_ps = mhpool.tile([128, 256], F32)

                cengs = [nc.vector, nc.scalar]
                # dots -> SBUF (batched, frees PSUM); garbage blocks unused
                for pr in range(4):
                    _cp(cengs[pr % 2], ds_sb[:, pr * 128:(pr + 1) * 128],
                        dots_ps[pr][:, :])
                ds3 = ds_sb.rearrange("p (b c) -> p b c", b=8)
                nc.vector.reduce_max(
                    negmax[:, :], ds3, axis=mybir.AxisListType.X, negate=True,
                )
                nm_bc = negmax.rearrange("p (b o) -> p b o", o=1)
                nc.vector.tensor_add(
                    shif.rearrange("p (b c) -> p b c", b=8), ds3,
                    nm_bc.broadcast_to([128, 8, 64]),
                )
                nc.scalar.activation(
                    exps[:, :], shif[:, :], mybir.ActivationFunctionType.Exp
                )
                nc.vector.reduce_sum(
                    rowsum[:, :], exps.rearrange("p (b c) -> p b c", b=8),
                    axis=mybir.AxisListType.X,
                )
                nc.vector.reciprocal(recip[:, :], rowsum[:, :])
                for h in range(HEADS):
                    nc.vector.tensor_scalar_mul(
                        wots[:, h * 64:(h + 1) * 64],
                        wot_sb[:, :],
                        recip[:, h:h + 1],
                    )
                for h in range(HEADS):
                    pr = h // 2
                    b = (h % 2) * 64
                    nc.tensor.matmul(
                        mh_ps[b:b + 64, pr * 64:(pr + 1) * 64],
                        lhsT=exps[b:b + 64,
                                  pr * 128 + b:pr * 128 + b + 64],
                        rhs=wots[b:b + 64, h * 64:(h + 1) * 64],
                        start=True,
                        stop=True,
                    )
                nc.vector.tensor_copy(mh_sb[:, :], mh_ps[:, :])

                ci = 0
                for pr in range(4):
                    for c0 in range(c0n):
                        outsb = outpool.tile([128, chunk_b], F32, name="outsb")
                        for s5 in range(s5n):
                            fp_ = finpool.tile([128, 512], F32, name="fp_")
                            n0 = pr * bl + c0 * chunk_b + s5 * 512
                            nc.tensor.matmul(
                                fp_[0:64, :],
                                lhsT=mh_sb[0:64, pr * 64:(pr + 1) * 64],
                                rhs=vdp[0:64, n0:n0 + 512],
                                start=True,
                                stop=True,
                            )
                            nc.tensor.matmul(
                                fp_[64:128, :],
                                lhsT=mh_sb[64:128, pr * 64:(pr + 1) * 64],
                                rhs=vdp[64:128, n0:n0 + 512],
                                start=True,
                                stop=True,
                            )
                            _cp(cengs[s5 % 2], outsb[:, s5 * 512:(s5 + 1) * 512],
                                fp_[:, :])
                        deng = nc.sync if ci % 2 == 0 else nc.scalar
                        deng.dma_start(
                            out=out_v[pr, :, :, c0 * chunk_b:(c0 + 1) * chunk_b],
                            in_=outsb[:, :],
                        )
                        ci += 1

    nc.compile()
    return nc


_NC_CACHE = {}


def _get_nc(hw=HW):
    if hw not in _NC_CACHE:
        _NC_CACHE[hw] = _build_kernel(hw)
    return _NC_CACHE[hw]


def _host_inputs(Wq, bq, Wk, bk, Wv, bv, Wo):
    scale = 64 ** -0.5
    wqkv = np.zeros((65, 256), np.float32)
    wqkv[:64, 0:64] = Wq.T * scale
    wqkv[64, 0:64] = bq * scale
    wqkv[:64, 64:128] = Wk.T
    wqkv[64, 64:128] = bk
    wqkv[:64, 128:192] = Wv.T
    wqkv[64, 128:192] = bv
    wot = np.concatenate([Wo.T, Wo.T], axis=0).astype(np.float32)
    ident = np.eye(128, dtype=np.float16)
    return wqkv, wot, ident


def kernel(x, Wq, bq, Wk, bk, Wv, bv, Wo):
    global LAST_RESULTS
    B = x.shape[0]
    hw = x.shape[2] * x.shape[3]
    nc = _get_nc(hw)
    wqkv, wot, ident = _host_inputs(Wq, bq, Wk, bk, Wv, bv, Wo)

    in_maps = []
    for bidx in range(B):
        x65 = np.empty((65, hw), np.float32)
        x65[:64] = x[bidx].reshape(64, hw)
        x65[64] = 1.0
        in_maps.append({"x": x65, "wqkv": wqkv, "wot": wot, "ident": ident})

    trace = bool(os.environ.get("KERNEL_TRACE"))
    res = run_bass_kernel_spmd(
        nc, in_maps, core_ids=list(range(B)), trace=trace
    )
    LAST_RESULTS = res
    out = np.stack(
        [res.results[bidx]["out"].reshape(64, HEADS, hw // HEADS)
         for bidx in range(B)]
    )
    return out


# revision 6
# speedup vs baseline: 1.6196x; 1.6196x over previous
"""Trainium2 Bass kernel for nn_Attention_5927054869144.

Channel-attention over [B=8, C=64, H=256, W=256] inputs. Data-parallel over
batch: one batch element per NeuronCore (8 cores), no collectives.

Per-core pipeline (x_b viewed as [64, 65536], 8 spatial blocks of 8192):
  1. x loaded fp32 via HWDGE (sync/scalar alternating) in 8 chunks of
     [65, 8, 1024] (4KB descriptors, sprayed over all 16 SDMA engines).
  2. GPSIMD casts each chunk fp32 -> fp16 (SBUF->SBUF; GPSIMD is otherwise
     idle and cannot touch PSUM anyway).
  3. qkvT projection with x-chunk stationary on the PE (fp16, N=192),
     evacuated fp32->fp16 into (r, h, i, alpha) slots on vector/scalar
     (softmax scale folded into Wq/bq host-side).
  4. Per-head-pair dots matmuls accumulated in PSUM over all 64 tile groups;
     v transposed to dim-partition layout via TensorE transposes.
  5. Batched softmax: one reduce_max, one broadcast-subtract, one fp16 Exp,
     one reduce_sum, one reciprocal folded into per-head copies of Wo^T;
     8 tiny mh matmuls.
  6. Final output = mh^T @ v_dp per 512-column chunk (fp16 operands),
     evacuated fp32; output DMA as simple [64, 4096] per-parity stores
     (single-strided APs so HWDGE sprays all 16 SDMA engines), round-robin
     over sync/scalar/gpsimd queues.
"""

import os
import sys

import numpy as np

for _p in ("/opt/trn_rl_repo", "/root/.axon_site/_ro/trn_rl_repo"):
    if os.path.isdir(_p) and _p not in sys.path:
        sys.path.insert(0, _p)

from concourse import bacc, mybir, tile  # noqa: E402
from concourse.bass_utils import run_bass_kernel_spmd  # noqa: E402

F32 = mybir.dt.float32
F16 = mybir.dt.float16

HEADS = 8
C = 64
HW = 65536          # 256*256 spatial positions per batch element
BL = HW // HEADS    # 8192, per-head block length
NE = 8              # input chunks (intra-block column ranges)
CHUNK_B = 4096      # phase-B output chunk columns per head

LAST_RESULTS = None


def _cp(eng, out, in_):
    if hasattr(eng, "tensor_copy"):
        eng.tensor_copy(out, in_)
    else:
        eng.copy(out, in_)


def _build_kernel(hw=HW):
    bl = hw // HEADS
    el = bl // NE           # 1024 intra-block cols per chunk
    tpe = el // 128         # 8 tile groups per chunk
    n_groups = bl // 128    # 64 total tile groups
    chunk_b = min(CHUNK_B, bl)
    s5n = chunk_b // 512
    c0n = bl // chunk_b

    nc = bacc.Bacc("TRN2", target_bir_lowering=False, debug=False)
    x_d = nc.dram_tensor("x", [65, hw], F32, kind="ExternalInput")
    wqkv_d = nc.dram_tensor("wqkv", [65, 192], F16, kind="ExternalInput")
    wot_d = nc.dram_tensor("wot", [128, 64], F32, kind="ExternalInput")
    ident_d = nc.dram_tensor("ident", [128, 128], F16, kind="ExternalInput")
    out_d = nc.dram_tensor("out", [64, hw], F32, kind="ExternalOutput")

    x_ap = x_d.ap()
    out_ap = out_d.ap()

    with tile.TileContext(nc) as tc:
        with (
            tc.tile_pool(name="consts", bufs=1) as cpool,
            tc.tile_pool(name="pers", bufs=1) as pers,
            tc.tile_pool(name="dotsp", bufs=1, space="PSUM") as dotspool,
        ):
            wqkv_sb = cpool.tile([65, 192], F16)
            wot_sb = cpool.tile([128, 64], F32)
            ident_sb = cpool.tile([128, 128], F16)
            nc.sync.dma_start(out=wqkv_sb[:, :], in_=wqkv_d.ap()[:, :])
            nc.scalar.dma_start(out=wot_sb[:, :], in_=wot_d.ap()[:, :])
            nc.sync.dma_start(out=ident_sb[:, :], in_=ident_d.ap()[:, :])

            # v in dim-partition layout: [d_even(0:64)|d_odd(64:128), pair, n]
            vdp = pers.tile([128, 4 * bl], F16)
            vdp_v = vdp.rearrange("p (r n) -> p r n", r=4)
            dots_ps = [
                dotspool.tile([128, 128], F32, name=f"dots{p}") for p in range(4)
            ]

            # ---------------- Phase A ----------------
            with (
                tc.tile_pool(name="xf", bufs=2) as xfpool,
                tc.tile_pool(name="xh", bufs=2) as xhpool,
                tc.tile_pool(name="slots", bufs=5) as slotpool,
                tc.tile_pool(name="projp", bufs=2, space="PSUM") as projpool,
                tc.tile_pool(name="vtrp", bufs=2, space="PSUM") as vtrpool,
            ):
                x_blk = x_ap.rearrange("p (i n) -> p i n", i=8)
                slots = {}

                def consume(g):
                    # dots + v-transpose for a group whose slot is fully evac'd
                    slot = slots.pop(g)
                    vt = vtrpool.tile([128, 512], F16, name="vt")
                    for pr in range(4):
                        qs = slot[:, 128 * pr: 128 * pr + 128]
                        ks = slot[:, 512 + 128 * pr: 512 + 128 * pr + 128]
                        vs = slot[:, 1024 + 128 * pr: 1024 + 128 * pr + 128]
                        nc.tensor.matmul(
                            dots_ps[pr][:, :],
                            lhsT=qs,
                            rhs=ks,
                            start=(g == 0),
                            stop=(g == n_groups - 1),
                        )
                        nc.tensor.transpose(
                            vt[:, pr * 128:(pr + 1) * 128], vs, ident_sb[:, :]
                        )
                    voff = g * 128
                    ve = nc.vector if g % 2 == 0 else nc.scalar
                    _cp(ve, vdp_v[:, :, voff:voff + 128], vt[:, :])

                for e in range(NE):
                    xf = xfpool.tile([65, 8 * el], F32, name="xf")
                    xf_v = xf.rearrange("p (i n) -> p i n", i=8)
                    deng = nc.sync if e % 2 == 0 else nc.scalar
                    deng.dma_start(
                        out=xf_v,
                        in_=x_blk[:, :, e * el:(e + 1) * el],
                    )
                    xh = xhpool.tile([65, 8 * el], F16, name="xh")
                    nc.gpsimd.tensor_copy(xh[:, :], xf[:, :])
                    xh_v = xh.rearrange("p (i n) -> p i n", i=8)
                    for t0 in range(tpe):
                        g = e * tpe + t0
                        # slot cols: r*512 + h*64 + i*8 + alpha
                        slot = slotpool.tile([128, 1536], F16, name="slot")
                        slot_sc = slot.rearrange(
                            "p (r h i a) -> p i r h a", r=3, h=8, i=8, a=8
                        )
                        slots[g] = slot
                        for ip in range(4):  # block pairs (2i, 2i+1)
                            pp = projpool.tile([128, 384], F32, name="pp")
                            for cc in range(2):
                                i = 2 * ip + cc
                                nc.tensor.matmul(
                                    pp[:, cc * 192:(cc + 1) * 192],
                                    lhsT=xh_v[:, i, t0 * 128:t0 * 128 + 128],
                                    rhs=wqkv_sb[:, :],
                                    start=True,
                                    stop=True,
                                )
                            src = pp.rearrange(
                                "p (i2 r h a) -> p i2 r h a", i2=2, r=3, h=8)
                            dst = slot_sc[:, 2 * ip:2 * ip + 2, :, :, :]
                            ce = nc.vector if ip % 2 == 0 else nc.scalar
                            _cp(ce, dst, src)
                        if g >= 3:
                            consume(g - 3)
                for g in (n_groups - 3, n_groups - 2, n_groups - 1):
                    consume(g)

            # ---------------- Softmax + output ----------------
            with (
                tc.tile_pool(name="smx", bufs=1) as smx,
                tc.tile_pool(name="mhp", bufs=1, space="PSUM") as mhpool,
                tc.tile_pool(name="finp", bufs=3, space="PSUM") as finpool,
                tc.tile_pool(name="outs", bufs=3) as outpool,
            ):
                ds_sb = smx.tile([128, 512], F32)
                shif = smx.tile([128, 512], F32)
                negmax = smx.tile([128, 8], F32)
                exps = smx.tile([128, 512], F16)
                rowsum = smx.tile([128, 8], F32)
                recip = smx.tile([128, 8], F32)
                wots = smx.tile([128, 512], F16)
                mh_sb = smx.tile([128, 256], F16)
                mh_ps = mhpool.tile([128, 256], F32)

                cengs = [nc.vector, nc.scalar]
                # dots -> SBUF (batched, frees PSUM); garbage blocks unused
                for pr in range(4):
                    _cp(cengs[pr % 2], ds_sb[:, pr * 128:(pr + 1) * 128],
                        dots_ps[pr][:, :])
                ds3 = ds_sb.rearrange("p (b c) -> p b c", b=8)
                nc.vector.reduce_max(
                    negmax[:, :], ds3, axis=mybir.AxisListType.X, negate=True,
                )
                nm_bc = negmax.rearrange("p (b o) -> p b o", o=1)
                nc.vector.tensor_add(
                    shif.rearrange("p (b c) -> p b c", b=8), ds3,
                    nm_bc.broadcast_to([128, 8, 64]),
                )
                nc.scalar.activation(
                    exps[:, :], shif[:, :], mybir.ActivationFunctionType.Exp
                )
                nc.vector.reduce_sum(
                    rowsum[:, :], exps.rearrange("p (b c) -> p b c", b=8),
                    axis=mybir.AxisListType.X,
                )
                nc.vector.reciprocal(recip[:, :], rowsum[:, :])
                for h in range(HEADS):
                    nc.vector.tensor_scalar_mul(
                        wots[:, h * 64:(h + 1) * 64],
                        wot_sb[:, :],
                        recip[:, h:h + 1],
                    )
                for h in range(HEADS):
                    pr = h // 2
                    b = (h % 2) * 64
                    nc.tensor.matmul(
                        mh_ps[b:b + 64, pr * 64:(pr + 1) * 64],
                        lhsT=exps[b:b + 64,
                                  pr * 128 + b:pr * 128 + b + 64],
                        rhs=wots[b:b + 64, h * 64:(h + 1) * 64],
                        start=True,
                        stop=True,
                    )
                nc.vector.tensor_copy(mh_sb[:, :], mh_ps[:, :])

                dma_engs = [nc.sync, nc.scalar, nc.gpsimd]
                ci = 0
                for pr in range(4):
                    for c0 in range(c0n):
                        outsb = outpool.tile([128, chunk_b], F32, name="outsb")
                        for s5 in range(s5n):
                            fp_ = finpool.tile([128, 512], F32, name="fp_")
                            n0 = pr * bl + c0 * chunk_b + s5 * 512
                            nc.tensor.matmul(
                                fp_[0:64, :],
                                lhsT=mh_sb[0:64, pr * 64:(pr + 1) * 64],
                                rhs=vdp[0:64, n0:n0 + 512],
                                start=True,
                                stop=True,
                            )
                            nc.tensor.matmul(
                                fp_[64:128, :],
                                lhsT=mh_sb[64:128, pr * 64:(pr + 1) * 64],
                                rhs=vdp[64:128, n0:n0 + 512],
                                start=True,
                                stop=True,
                            )
                            _cp(cengs[s5 % 2], outsb[:, s5 * 512:(s5 + 1) * 512],
                                fp_[:, :])
                        # two simple single-strided stores per chunk: head
                        # h = 2*pr + s lives at DRAM cols h*bl + [c0*chunk_b ..)
                        for s in range(2):
                            col0 = (2 * pr + s) * bl + c0 * chunk_b
                            dma_engs[ci % 3].dma_start(
                                out=out_ap[:, col0:col0 + chunk_b],
                                in_=outsb[s * 64:(s + 1) * 64, :],
                            )
                            ci += 1

    nc.compile()
    return nc


_NC_CACHE = {}


def _get_nc(hw=HW):
    if hw not in _NC_CACHE:
        _NC_CACHE[hw] = _build_kernel(hw)
    return _NC_CACHE[hw]


def _host_inputs(Wq, bq, Wk, bk, Wv, bv, Wo):
    scale = 64 ** -0.5
    wqkv = np.zeros((65, 192), np.float16)
    wqkv[:64, 0:64] = (Wq.T * scale).astype(np.float16)
    wqkv[64, 0:64] = (bq * scale).astype(np.float16)
    wqkv[:64, 64:128] = Wk.T.astype(np.float16)
    wqkv[64, 64:128] = bk.astype(np.float16)
    wqkv[:64, 128:192] = Wv.T.astype(np.float16)
    wqkv[64, 128:192] = bv.astype(np.float16)
    # kernel uses d = i*8 + alpha ordering; original d = alpha*8 + i
    pi = np.array([(c % 8) * 8 + c // 8 for c in range(64)])
    wotp = Wo.T[pi]
    wot = np.concatenate([wotp, wotp], axis=0).astype(np.float32)
    ident = np.eye(128, dtype=np.float16)
    return wqkv, wot, ident


def kernel(x, Wq, bq, Wk, bk, Wv, bv, Wo):
    global LAST_RESULTS
    B = x.shape[0]
    hw = x.shape[2] * x.shape[3]
    nc = _get_nc(hw)
    wqkv, wot, ident = _host_inputs(Wq, bq, Wk, bk, Wv, bv, Wo)

    in_maps = []
    for bidx in range(B):
        x65 = np.empty((65, hw), np.float32)
        x65[:64] = x[bidx].reshape(64, hw)
        x65[64] = 1.0
        in_maps.append({"x": x65, "wqkv": wqkv, "wot": wot, "ident": ident})

    trace = bool(os.environ.get("KERNEL_TRACE"))
    res = run_bass_kernel_spmd(
        nc, in_maps, core_ids=list(range(B)), trace=trace
    )
    LAST_RESULTS = res
    out = np.stack(
        [res.results[bidx]["out"].reshape(64, HEADS, hw // HEADS)
         for bidx in range(B)]
    )
    return out


# revision 7
# speedup vs baseline: 2.6826x; 1.6563x over previous
"""Trainium2 Bass kernel for nn_Attention_5927054869144.

Channel-attention over [B=8, C=64, H=256, W=256] inputs. Data-parallel over
batch: one batch element per NeuronCore (8 cores), no collectives.

Per-core pipeline (x_b viewed as [64, 65536], 8 spatial blocks of 8192):
  1. x loaded via SWDGE cast-DMA (fp32 -> fp16 in the DMA, ~245 GB/s
     measured) in 8 chunks of [65, 8, 1024]; HWDGE rings stay free for
     the output stores.
  2. qkvT projection with x-chunk stationary on the PE (fp16, N=192),
     evacuated fp32->fp16 into (r, h, i, alpha) slots on vector/scalar
     (softmax scale folded into Wq/bq host-side).
  4. Per-head-pair dots matmuls accumulated in PSUM over all 64 tile groups;
     v transposed to dim-partition layout via TensorE transposes.
  5. Batched softmax: one reduce_max, one broadcast-subtract, one fp16 Exp,
     one reduce_sum, one reciprocal folded into per-head copies of Wo^T;
     8 tiny mh matmuls.
  6. Final output = mh^T @ v_dp per 512-column chunk (fp16 operands),
     evacuated fp32; output DMA as simple [64, 4096] per-parity stores
     (single-strided APs so HWDGE sprays all 16 SDMA engines), round-robin
     over sync/scalar/gpsimd queues.
"""

import os
import sys

import numpy as np

for _p in ("/opt/trn_rl_repo", "/root/.axon_site/_ro/trn_rl_repo"):
    if os.path.isdir(_p) and _p not in sys.path:
        sys.path.insert(0, _p)

from concourse import bacc, mybir, tile  # noqa: E402
from concourse.bass_utils import run_bass_kernel_spmd  # noqa: E402

F32 = mybir.dt.float32
F16 = mybir.dt.float16

HEADS = 8
C = 64
HW = 65536          # 256*256 spatial positions per batch element
BL = HW // HEADS    # 8192, per-head block length
NE = 8              # input chunks (intra-block column ranges)
CHUNK_B = 4096      # phase-B output chunk columns per head

LAST_RESULTS = None


def _cp(eng, out, in_):
    if hasattr(eng, "tensor_copy"):
        eng.tensor_copy(out, in_)
    else:
        eng.copy(out, in_)


def _build_kernel(hw=HW):
    bl = hw // HEADS
    el = bl // NE           # 1024 intra-block cols per chunk
    tpe = el // 128         # 8 tile groups per chunk
    n_groups = bl // 128    # 64 total tile groups
    chunk_b = min(CHUNK_B, bl)
    s5n = chunk_b // 512
    c0n = bl // chunk_b

    nc = bacc.Bacc("TRN2", target_bir_lowering=False, debug=False)
    x_d = nc.dram_tensor("x", [65, hw], F32, kind="ExternalInput")
    wqkv_d = nc.dram_tensor("wqkv", [65, 192], F16, kind="ExternalInput")
    wot_d = nc.dram_tensor("wot", [128, 64], F32, kind="ExternalInput")
    ident_d = nc.dram_tensor("ident", [128, 128], F16, kind="ExternalInput")
    out_d = nc.dram_tensor("out", [64, hw], F32, kind="ExternalOutput")

    x_ap = x_d.ap()
    out_ap = out_d.ap()

    with tile.TileContext(nc) as tc:
        with (
            tc.tile_pool(name="consts", bufs=1) as cpool,
            tc.tile_pool(name="pers", bufs=1) as pers,
            tc.tile_pool(name="dotsp", bufs=1, space="PSUM") as dotspool,
        ):
            wqkv_sb = cpool.tile([65, 192], F16)
            wot_sb = cpool.tile([128, 64], F32)
            ident_sb = cpool.tile([128, 128], F16)
            nc.sync.dma_start(out=wqkv_sb[:, :], in_=wqkv_d.ap()[:, :])
            nc.scalar.dma_start(out=wot_sb[:, :], in_=wot_d.ap()[:, :])
            nc.sync.dma_start(out=ident_sb[:, :], in_=ident_d.ap()[:, :])

            # v in dim-partition layout: [d_even(0:64)|d_odd(64:128), pair, n]
            vdp = pers.tile([128, 4 * bl], F16)
            vdp_v = vdp.rearrange("p (r n) -> p r n", r=4)
            dots_ps = [
                dotspool.tile([128, 128], F32, name=f"dots{p}") for p in range(4)
            ]

            # ---------------- Phase A ----------------
            with (
                tc.tile_pool(name="xh", bufs=2) as xhpool,
                tc.tile_pool(name="slots", bufs=5) as slotpool,
                tc.tile_pool(name="projp", bufs=2, space="PSUM") as projpool,
                tc.tile_pool(name="vtrp", bufs=2, space="PSUM") as vtrpool,
            ):
                x_blk = x_ap.rearrange("p (i n) -> p i n", i=8)
                slots = {}

                def consume(g):
                    # dots + v-transpose for a group whose slot is fully evac'd
                    slot = slots.pop(g)
                    vt = vtrpool.tile([128, 512], F16, name="vt")
                    for pr in range(4):
                        qs = slot[:, 128 * pr: 128 * pr + 128]
                        ks = slot[:, 512 + 128 * pr: 512 + 128 * pr + 128]
                        vs = slot[:, 1024 + 128 * pr: 1024 + 128 * pr + 128]
                        nc.tensor.matmul(
                            dots_ps[pr][:, :],
                            lhsT=qs,
                            rhs=ks,
                            start=(g == 0),
                            stop=(g == n_groups - 1),
                        )
                        nc.tensor.transpose(
                            vt[:, pr * 128:(pr + 1) * 128], vs, ident_sb[:, :]
                        )
                    voff = g * 128
                    ve = nc.vector if g % 2 == 0 else nc.scalar
                    _cp(ve, vdp_v[:, :, voff:voff + 128], vt[:, :])

                for e in range(NE):
                    xh = xhpool.tile([65, 8 * el], F16, name="xh")
                    xh_v = xh.rearrange("p (i n) -> p i n", i=8)
                    nc.gpsimd.dma_start(
                        out=xh_v,
                        in_=x_blk[:, :, e * el:(e + 1) * el],
                    )
                    for t0 in range(tpe):
                        g = e * tpe + t0
                        # slot cols: r*512 + h*64 + i*8 + alpha
                        slot = slotpool.tile([128, 1536], F16, name="slot")
                        slot_sc = slot.rearrange(
                            "p (r h i a) -> p i r h a", r=3, h=8, i=8, a=8
                        )
                        slots[g] = slot
                        for ip in range(4):  # block pairs (2i, 2i+1)
                            pp = projpool.tile([128, 384], F32, name="pp")
                            for cc in range(2):
                                i = 2 * ip + cc
                                nc.tensor.matmul(
                                    pp[:, cc * 192:(cc + 1) * 192],
                                    lhsT=xh_v[:, i, t0 * 128:t0 * 128 + 128],
                                    rhs=wqkv_sb[:, :],
                                    start=True,
                                    stop=True,
                                )
                            src = pp.rearrange(
                                "p (i2 r h a) -> p i2 r h a", i2=2, r=3, h=8)
                            dst = slot_sc[:, 2 * ip:2 * ip + 2, :, :, :]
                            ce = nc.vector if ip % 2 == 0 else nc.scalar
                            _cp(ce, dst, src)
                        if g >= 3:
                            consume(g - 3)
                for g in (n_groups - 3, n_groups - 2, n_groups - 1):
                    consume(g)

            # ---------------- Softmax + output ----------------
            with (
                tc.tile_pool(name="smx", bufs=1) as smx,
                tc.tile_pool(name="mhp", bufs=1, space="PSUM") as mhpool,
                tc.tile_pool(name="finp", bufs=3, space="PSUM") as finpool,
                tc.tile_pool(name="outs", bufs=3) as outpool,
            ):
                ds_sb = smx.tile([128, 512], F32)
                shif = smx.tile([128, 512], F32)
                negmax = smx.tile([128, 8], F32)
                exps = smx.tile([128, 512], F16)
                rowsum = smx.tile([128, 8], F32)
                recip = smx.tile([128, 8], F32)
                wots = smx.tile([128, 512], F16)
                mh_sb = smx.tile([128, 256], F16)
                mh_ps = mhpool.tile([128, 256], F32)

                cengs = [nc.vector, nc.scalar]
                # dots -> SBUF (batched, frees PSUM); garbage blocks unused
                for pr in range(4):
                    _cp(cengs[pr % 2], ds_sb[:, pr * 128:(pr + 1) * 128],
                        dots_ps[pr][:, :])
                ds3 = ds_sb.rearrange("p (b c) -> p b c", b=8)
                nc.vector.reduce_max(
                    negmax[:, :], ds3, axis=mybir.AxisListType.X, negate=True,
                )
                nm_bc = negmax.rearrange("p (b o) -> p b o", o=1)
                nc.vector.tensor_add(
                    shif.rearrange("p (b c) -> p b c", b=8), ds3,
                    nm_bc.broadcast_to([128, 8, 64]),
                )
                nc.scalar.activation(
                    exps[:, :], shif[:, :], mybir.ActivationFunctionType.Exp
                )
                nc.vector.reduce_sum(
                    rowsum[:, :], exps.rearrange("p (b c) -> p b c", b=8),
                    axis=mybir.AxisListType.X,
                )
                nc.vector.reciprocal(recip[:, :], rowsum[:, :])
                for h in range(HEADS):
                    nc.vector.tensor_scalar_mul(
                        wots[:, h * 64:(h + 1) * 64],
                        wot_sb[:, :],
                        recip[:, h:h + 1],
                    )
                for h in range(HEADS):
                    pr = h // 2
                    b = (h % 2) * 64
                    nc.tensor.matmul(
                        mh_ps[b:b + 64, pr * 64:(pr + 1) * 64],
                        lhsT=exps[b:b + 64,
                                  pr * 128 + b:pr * 128 + b + 64],
                        rhs=wots[b:b + 64, h * 64:(h + 1) * 64],
                        start=True,
                        stop=True,
                    )
                nc.vector.tensor_copy(mh_sb[:, :], mh_ps[:, :])

                dma_engs = [nc.sync, nc.scalar, nc.gpsimd]
                ci = 0
                for pr in range(4):
                    for c0 in range(c0n):
                        outsb = outpool.tile([128, chunk_b], F32, name="outsb")
                        for s5 in range(s5n):
                            fp_ = finpool.tile([128, 512], F32, name="fp_")
                            n0 = pr * bl + c0 * chunk_b + s5 * 512
                            nc.tensor.matmul(
                                fp_[0:64, :],
                                lhsT=mh_sb[0:64, pr * 64:(pr + 1) * 64],
                                rhs=vdp[0:64, n0:n0 + 512],
                                start=True,
                                stop=True,
                            )
                            nc.tensor.matmul(
                                fp_[64:128, :],
                                lhsT=mh_sb[64:128, pr * 64:(pr + 1) * 64],
                                rhs=vdp[64:128, n0:n0 + 512],
                                start=True,
                                stop=True,
                            )
                            _cp(cengs[s5 % 2], outsb[:, s5 * 512:(s5 + 1) * 512],
                                fp_[:, :])
                        # two simple single-strided stores per chunk: head
                        # h = 2*pr + s lives at DRAM cols h*bl + [c0*chunk_b ..)
                        for s in range(2):
                            col0 = (2 * pr + s) * bl + c0 * chunk_b
                            dma_engs[ci % 3].dma_start(
                                out=out_ap[:, col0:col0 + chunk_b],
                                in_=outsb[s * 64:(s + 1) * 64, :],
                            )
                            ci += 1

    nc.compile()
    return nc


_NC_CACHE = {}


def _get_nc(hw=HW):
    if hw not in _NC_CACHE:
        _NC_CACHE[hw] = _build_kernel(hw)
    return _NC_CACHE[hw]


def _host_inputs(Wq, bq, Wk, bk, Wv, bv, Wo):
    scale = 64 ** -0.5
    wqkv = np.zeros((65, 192), np.float16)
    wqkv[:64, 0:64] = (Wq.T * scale).astype(np.float16)
    wqkv[64, 0:64] = (bq * scale).astype(np.float16)
    wqkv[:64, 64:128] = Wk.T.astype(np.float16)
    wqkv[64, 64:128] = bk.astype(np.float16)
    wqkv[:64, 128:192] = Wv.T.astype(np.float16)
    wqkv[64, 128:192] = bv.astype(np.float16)
    # kernel uses d = i*8 + alpha ordering; original d = alpha*8 + i
    pi = np.array([(c % 8) * 8 + c // 8 for c in range(64)])
    wotp = Wo.T[pi]
    wot = np.concatenate([wotp, wotp], axis=0).astype(np.float32)
    ident = np.eye(128, dtype=np.float16)
    return wqkv, wot, ident


def kernel(x, Wq, bq, Wk, bk, Wv, bv, Wo):
    global LAST_RESULTS
    B = x.shape[0]
    hw = x.shape[2] * x.shape[3]
    nc = _get_nc(hw)
    wqkv, wot, ident = _host_inputs(Wq, bq, Wk, bk, Wv, bv, Wo)

    in_maps = []
    for bidx in range(B):
        x65 = np.empty((65, hw), np.float32)
        x65[:64] = x[bidx].reshape(64, hw)
        x65[64] = 1.0
        in_maps.append({"x": x65, "wqkv": wqkv, "wot": wot, "ident": ident})

    trace = bool(os.environ.get("KERNEL_TRACE"))
    res = run_bass_kernel_spmd(
        nc, in_maps, core_ids=list(range(B)), trace=trace
    )
    LAST_RESULTS = res
    out = np.stack(
        [res.results[bidx]["out"].reshape(64, HEADS, hw // HEADS)
         for bidx in range(B)]
    )
    return out
